# revision 8
# baseline (speedup 1.0000x reference)
"""Trainium2 Bass kernel for gnn_message_passing nn_CNNTest_10299331576114.

Strategy (V=100000 vertices sharded over 8 cores, 12500 each):

Stage 1 (NEFF-1): per core, gather g = vp[nb1] (12500x32 scalar indirect-DMA
gather), conv-k3 over the neighbor axis as a small banded matmul (host-packed
[33,32] matrix incl. bias row), relu, sum over neighbors -> h_raw shard
(mean's 1/32 is folded into downstream matrices).

Host: concat shards, build zero-padded gather table hp[100002].

Stage 2 (NEFF-2): per core, for each (v,j) gather the TRIPLE
(h[u-1], h[u], h[u+1]) = hp[u..u+2] where u = nb2[v,j] (12 B per index instead
of a 128 B f1 row - all the f1/conv math is linear pre-relu, so it is folded
into one host-packed [97,1024] matrix). Then:
  C = [T|1] @ Mbig   (PE),  relu (ACT),  h2 = sum_j (DVE reduce)
  h2^T written into a [33, 12502] vertex strip (PE transpose), ones row for
  biases, edge columns masked by a per-core input mask.
  f2^T = sum_k w2k^T @ strip_shift_k (PE, PSUM accum), logits = [f2|1]^T@wfcb,
  softmax via ACT exp with accumulated row-sum + DVE reciprocal/scale.
"""

import time

import numpy as np

import concourse.bacc as bacc
import concourse.mybir as mybir
import concourse.tile as tile
from concourse import bass
from concourse.bass import IndirectOffsetOnAxis
from concourse.bass_utils import run_bass_kernel_spmd
from concourse.masks import make_identity

F32 = mybir.dt.float32
I32 = mybir.dt.int32
AX = mybir.AxisListType
ALU = mybir.AluOpType
ACTF = mybir.ActivationFunctionType

V = 100000
N = 32
NCORES = 8
VC = V // NCORES          # 12500
P = 125                   # vertices per tile (partition dim)
T1 = VC // P              # 100 tiles per core
EXT = VC + 2              # stage-2 extended range (one halo vertex each side)
TA = T1 + 1               # 101 stage-2 gather tiles (last overlaps)

_CACHE = {}
TIMES = {}


def _build_stage1(repeat=1):
    nc = bacc.Bacc("TRN2", target_bir_lowering=False, debug=False,
                   num_devices=NCORES)
    vp = nc.dram_tensor("vp", [V], F32, kind="ExternalInput")
    nb1 = nc.dram_tensor("nb1", [VC, N], I32, kind="ExternalInput")
    a1 = nc.dram_tensor("a1", [N + 1, N], F32, kind="ExternalInput")
    hsh = nc.dram_tensor("hsh", [VC], F32, kind="ExternalOutput")

    with tile.TileContext(nc) as tc:
        with (
            tc.tile_pool(name="const", bufs=1) as cp,
            tc.tile_pool(name="io", bufs=4) as iop,
            tc.tile_pool(name="work", bufs=4) as wp,
            tc.tile_pool(name="hc", bufs=1) as hcp,
            tc.tile_pool(name="ps", bufs=2, space="PSUM") as psp,
            tc.tile_pool(name="psb", bufs=1, space="PSUM") as psb,
        ):
            ident = cp.tile([128, 128], F32)
            make_identity(nc, ident[:])
            a1t = cp.tile([N + 1, N], F32)
            nc.sync.dma_start(a1t[:], a1[:])
            hcol = hcp.tile([P, T1], F32)

            rep = tc.For_i(0, repeat, 1) if repeat > 1 else None
            if rep is not None:
                rep.__enter__()
            t = 0
            for nb_batch in ([8] * (T1 // 8) + ([T1 % 8] if T1 % 8 else [])):
                it = iop.tile([P, N * nb_batch], I32, tag="idx")
                nc.sync.dma_start(
                    it[:].rearrange("p (b n) -> p b n", n=N),
                    nb1[P * t:P * (t + nb_batch), :].rearrange(
                        "(b p) n -> p b n", p=P))
                g = wp.tile([P, N * nb_batch], F32, tag="g")
                nc.gpsimd.indirect_dma_start(
                    out=g[:], out_offset=None, in_=vp[:, None],
                    in_offset=IndirectOffsetOnAxis(ap=it[:], axis=0))
                for b in range(nb_batch):
                    gtp = psp.tile([N, P], F32, tag="gt")
                    nc.tensor.transpose(gtp[:], g[:, N * b:N * (b + 1)],
                                        ident[:P, :P])
                    gt = wp.tile([N + 1, P], F32, tag="gts")
                    nc.vector.tensor_copy(gt[:N, :], gtp[:])
                    nc.vector.memset(gt[N:N + 1, :], 1.0)
                    c1p = psp.tile([P, N], F32, tag="c1")
                    nc.tensor.matmul(c1p[:], lhsT=gt[:], rhs=a1t[:],
                                     start=True, stop=True)
                    r = wp.tile([P, N], F32, tag="r")
                    nc.scalar.activation(r[:], c1p[:], ACTF.Relu)
                    nc.vector.reduce_sum(hcol[:, t:t + 1], r[:], axis=AX.X)
                    t += 1

            if rep is not None:
                rep.__exit__(None, None, None)
            htp = psb.tile([T1, P], F32)
            nc.tensor.transpose(htp[:], hcol[:], ident[:P, :P])
            hst = wp.tile([T1, P], F32, tag="hst")
            nc.vector.tensor_copy(hst[:], htp[:])
            nc.sync.dma_start(
                hsh[:].rearrange("(t p) -> t p", p=P), hst[:])
    nc.finalize()
    return nc


def _build_stage2(repeat=1):
    nc = bacc.Bacc("TRN2", target_bir_lowering=False, debug=False,
                   num_devices=NCORES)
    hp = nc.dram_tensor("hp", [V + 2], F32, kind="ExternalInput")
    nb2e = nc.dram_tensor("nb2e", [EXT, N], I32, kind="ExternalInput")
    mbig = nc.dram_tensor("mbig", [97, 1024], F32, kind="ExternalInput")
    w2k3 = nc.dram_tensor("w2k3", [3, 33, 64], F32, kind="ExternalInput")
    wfcb = nc.dram_tensor("wfcb", [65, 512], F32, kind="ExternalInput")
    mask2 = nc.dram_tensor("mask2", [32, 2], F32, kind="ExternalInput")
    out = nc.dram_tensor("out", [VC, 512], F32, kind="ExternalOutput")

    with tile.TileContext(nc) as tc:
        with (
            tc.tile_pool(name="const", bufs=1) as cp,
            tc.tile_pool(name="strip", bufs=1) as sp,
            tc.tile_pool(name="io", bufs=4) as iop,
            tc.tile_pool(name="work", bufs=4) as wp,
            tc.tile_pool(name="big", bufs=3) as bp,
            tc.tile_pool(name="psc", bufs=2, space="PSUM") as psc,
            tc.tile_pool(name="pst", bufs=2, space="PSUM") as pst,
            tc.tile_pool(name="psl", bufs=2, space="PSUM") as psl,
        ):
            ident = cp.tile([128, 128], F32)
            make_identity(nc, ident[:])
            mbigt = cp.tile([97, 1024], F32)
            nc.sync.dma_start(mbigt[:], mbig[:])
            w2kt = []
            for k in range(3):
                w2tile = cp.tile([33, 64], F32, tag=f"w2k{k}")
                nc.sync.dma_start(w2tile[:], w2k3[k])
                w2kt.append(w2tile)
            wfcbt = cp.tile([65, 512], F32)
            nc.sync.dma_start(wfcbt[:], wfcb[:])
            m2t = cp.tile([32, 2], F32)
            nc.sync.dma_start(m2t[:], mask2[:])

            strip = sp.tile([33, EXT], F32)
            nc.vector.memset(strip[32:33, :], 1.0)

            def phase_a_batch(t0, nb_batch):
                it = iop.tile([P, N * nb_batch], I32, tag="idx")
                nc.sync.dma_start(
                    it[:].rearrange("p (b n) -> p b n", n=N),
                    nb2e[P * t0:P * (t0 + nb_batch), :].rearrange(
                        "(b p) n -> p b n", p=P))
                tt = wp.tile([P, 3 * N * nb_batch], F32, tag="tt")
                nc.gpsimd.indirect_dma_start(
                    out=tt[:], out_offset=None, in_=hp[:, None],
                    in_offset=IndirectOffsetOnAxis(ap=it[:], axis=0))
                for b in range(nb_batch):
                    phase_a_tail(t0 + b, P * (t0 + b),
                                 tt[:, 96 * b:96 * (b + 1)])

            def phase_a(t):
                ot = min(P * t, EXT - P)
                it = iop.tile([P, N], I32, tag="idx")
                nc.sync.dma_start(it[:], nb2e[ot:ot + P, :])
                tt = wp.tile([P, 3 * N], F32, tag="tt")
                nc.gpsimd.indirect_dma_start(
                    out=tt[:], out_offset=None, in_=hp[:, None],
                    in_offset=IndirectOffsetOnAxis(ap=it[:], axis=0))
                phase_a_tail(t, ot, tt[:])

            def phase_a_tail(t, ot, tt_ap):
                ttp = pst.tile([96, P], F32, tag="tp")
                nc.tensor.transpose(ttp[:], tt_ap, ident[:P, :P])
                tts = wp.tile([97, P], F32, tag="tts")
                nc.vector.tensor_copy(tts[:96, :], ttp[:])
                nc.vector.memset(tts[96:97, :], 1.0)
                cps = psc.tile([P, 1024], F32, tag="c")
                nc.tensor.matmul(cps[:, 0:512], lhsT=tts[:],
                                 rhs=mbigt[:, 0:512], start=True, stop=True)
                nc.tensor.matmul(cps[:, 512:1024], lhsT=tts[:],
                                 rhs=mbigt[:, 512:1024], start=True, stop=True)
                cr = bp.tile([P, 1024], F32, tag="cr")
                nc.scalar.activation(cr[:], cps[:], ACTF.Relu)
                h2 = wp.tile([P, N], F32, tag="h2")
                nc.vector.reduce_sum(
                    h2[:], cr[:].rearrange("p (c j) -> p c j", j=32),
                    axis=AX.X)
                h2p = pst.tile([N, P], F32, tag="tp")
                nc.tensor.transpose(h2p[:], h2[:], ident[:P, :P])
                nc.vector.tensor_copy(strip[0:32, ot:ot + P], h2p[:])
                if t == 0:
                    nc.vector.tensor_tensor(
                        out=strip[0:32, 0:1], in0=strip[0:32, 0:1],
                        in1=m2t[:, 0:1], op=ALU.mult)
                if t == TA - 1:
                    nc.vector.tensor_tensor(
                        out=strip[0:32, EXT - 1:EXT],
                        in0=strip[0:32, EXT - 1:EXT],
                        in1=m2t[:, 1:2], op=ALU.mult)

            def phase_b(t):
                f2p = pst.tile([64, P], F32, tag="tp")
                for k in range(3):
                    nc.tensor.matmul(
                        f2p[:], lhsT=w2kt[k][:],
                        rhs=strip[:, P * t + k:P * t + k + P],
                        start=(k == 0), stop=(k == 2))
                f2s = wp.tile([65, P], F32, tag="f2s")
                nc.vector.tensor_copy(f2s[:64, :], f2p[:])
                nc.vector.memset(f2s[64:65, :], 1.0)
                lgp = psl.tile([P, 512], F32, tag="lg")
                nc.tensor.matmul(lgp[:], lhsT=f2s[:], rhs=wfcbt[:],
                                 start=True, stop=True)
                e = bp.tile([P, 512], F32, tag="e")
                ssum = wp.tile([P, 1], F32, tag="ss")
                nc.scalar.activation(e[:], lgp[:], ACTF.Exp,
                                     accum_out=ssum[:])
                rinv = wp.tile([P, 1], F32, tag="ri")
                nc.vector.reciprocal(rinv[:], ssum[:])
                o = bp.tile([P, 512], F32, tag="o")
                nc.vector.tensor_scalar(out=o[:], in0=e[:], scalar1=rinv[:],
                                        scalar2=None, op0=ALU.mult)
                nc.sync.dma_start(out[bass.ts(t, P), :], o[:])

            rep = tc.For_i(0, repeat, 1) if repeat > 1 else None
            if rep is not None:
                rep.__enter__()
            done_b = 0
            done_a = 0
            for nb_batch in ([8] * (T1 // 8) + ([T1 % 8] if T1 % 8 else [])):
                phase_a_batch(done_a, nb_batch)
                done_a += nb_batch
                while done_b + 1 < done_a:
                    phase_b(done_b)
                    done_b += 1
            phase_a(TA - 1)   # overlapping last tile
            while done_b < T1:
                phase_b(done_b)
                done_b += 1
            if rep is not None:
                rep.__exit__(None, None, None)
    nc.finalize()
    return nc


def _host_mats(wv1, bv1, w1, b1, wv2, bv2, w2, b2, wfc, bfc):
    w1m = w1[:, 0, :].astype(np.float32)                    # [32, 3]
    a1 = np.zeros((N + 1, N), np.float32)                   # stage-1 conv
    for j in range(N):
        for dj in range(3):
            jp = j - 1 + dj
            if 0 <= jp < N:
                a1[jp, j] = wv1[dj]
    a1[N, :] = bv1[0]

    mbig = np.zeros((97, 1024), np.float32)
    cidx = np.arange(32) * 32
    for j in range(32):
        for dj in range(3):
            jp = j - 1 + dj
            if 0 <= jp < 32:
                for dk in range(3):
                    mbig[jp * 3 + dk, cidx + j] = wv2[dj] * w1m[:, dk] / 32.0
    for j in range(32):
        s = sum(wv2[dj] for dj in range(3) if 0 <= j - 1 + dj < 32)
        mbig[96, cidx + j] = bv2[0] + b1 * s

    w2k3 = np.zeros((3, 33, 64), np.float32)
    for k in range(3):
        w2k3[k, :32, :] = w2[:, :, k].T / 32.0
    w2k3[0, 32, :] = b2                                     # bias only on k=0

    wfcb = np.zeros((65, 512), np.float32)
    wfcb[:64] = wfc.T
    wfcb[64] = bfc
    return a1, mbig, w2k3, wfcb


def kernel(vp, nb1, nb2, wv1, bv1, w1, b1, wv2, bv2, w2, b2, wfc, bfc):
    vp = np.ascontiguousarray(np.asarray(vp, dtype=np.float32))
    nb1 = np.ascontiguousarray(np.asarray(nb1).astype(np.int32))
    nb2 = np.ascontiguousarray(np.asarray(nb2).astype(np.int32))
    wv1 = np.asarray(wv1, np.float32); bv1 = np.asarray(bv1, np.float32)
    w1 = np.asarray(w1, np.float32); b1 = np.asarray(b1, np.float32)
    wv2 = np.asarray(wv2, np.float32); bv2 = np.asarray(bv2, np.float32)
    w2 = np.asarray(w2, np.float32); b2 = np.asarray(b2, np.float32)
    wfc = np.asarray(wfc, np.float32); bfc = np.asarray(bfc, np.float32)

    a1, mbig, w2k3, wfcb = _host_mats(wv1, bv1, w1, b1, wv2, bv2, w2, b2,
                                      wfc, bfc)

    if "s1" not in _CACHE:
        _CACHE["s1"] = _build_stage1()
    if "s2" not in _CACHE:
        _CACHE["s2"] = _build_stage2()

    core_ids = list(range(NCORES))

    # ---- stage 1 ----
    in1 = [{"vp": vp, "nb1": nb1[VC * c:VC * (c + 1)], "a1": a1}
           for c in range(NCORES)]
    t0 = time.time()
    res1 = run_bass_kernel_spmd(_CACHE["s1"], in1, core_ids=core_ids)
    TIMES["stage1_wall"] = time.time() - t0
    hp = np.zeros(V + 2, np.float32)
    for c in range(NCORES):
        hp[1 + VC * c:1 + VC * (c + 1)] = res1.results[c]["hsh"]

    # ---- stage 2 ----
    in2 = []
    for c in range(NCORES):
        vstart = VC * c
        nb2e = np.zeros((EXT, N), np.int32)
        lo = max(vstart - 1, 0)
        hi = min(vstart + VC + 1, V)
        nb2e[lo - (vstart - 1):hi - (vstart - 1)] = nb2[lo:hi]
        mask2 = np.ones((32, 2), np.float32)
        if c == 0:
            mask2[:, 0] = 0.0
        if c == NCORES - 1:
            mask2[:, 1] = 0.0
        in2.append({"hp": hp, "nb2e": nb2e, "mbig": mbig, "w2k3": w2k3,
                    "wfcb": wfcb, "mask2": mask2})
    t0 = time.time()
    res2 = run_bass_kernel_spmd(_CACHE["s2"], in2, core_ids=core_ids)
    TIMES["stage2_wall"] = time.time() - t0
    return np.concatenate([res2.results[c]["out"] for c in range(NCORES)],
                          axis=0)


# revision 18
# speedup vs baseline: 7351.7248x; 7351.7248x over previous
"""Trainium2 Bass kernel for gnn_message_passing nn_CNNTest_10299331576114.

Strategy (V=100000 vertices sharded over 8 cores, 12500 each):

Stage 1 (NEFF-1): per core, gather g = vp[nb1] (12500x32 scalar indirect-DMA
gather), conv-k3 over the neighbor axis as a small banded matmul (host-packed
[33,32] matrix incl. bias row), relu, sum over neighbors -> h_raw shard
(mean's 1/32 is folded into downstream matrices).

Host: concat shards, build zero-padded gather table hp[100002].

Stage 2 (NEFF-2): per core, for each (v,j) gather the TRIPLE
(h[u-1], h[u], h[u+1]) = hp[u..u+2] where u = nb2[v,j] (12 B per index instead
of a 128 B f1 row - all the f1/conv math is linear pre-relu, so it is folded
into one host-packed [97,1024] matrix). Gathers are batched 8 vertex-tiles per
indirect-DMA instruction (the ~0.34 ns/descriptor SWDGE generation rate is the
hard floor; instruction fixed costs are amortized). Then per 125-vertex tile:
  C = [T|1] @ Mbig (PE, 2 matmuls), relu (ACT), h2 = sum_j (DVE reduce),
  h2^T -> [32, 12502] vertex strip (PE transposes batched 4 tiles wide).
A 3-row-shifted copy `strip3` [96, EXT] makes f2^T a SINGLE constant-weight
matmul per 500 vertices (PSUM accumulation over the vertex shift is folded
into the stacked [96,64] weight matrix; b2's contribution is folded into the
final bias row bfc' = bfc + wfc @ b2). logits = [f2|1]^T @ wfcb, softmax via
ACT exp with accumulated row-sum + DVE reciprocal/scale. Edge halo columns are
masked by a per-core {0,1} input so one SPMD NEFF serves all cores.
"""

import time

import numpy as np

import concourse.bacc as bacc
import concourse.mybir as mybir
import concourse.tile as tile
from concourse import bass
from concourse.bass import IndirectOffsetOnAxis
from concourse.bass_utils import run_bass_kernel_spmd
from concourse.masks import make_identity

F32 = mybir.dt.float32
I32 = mybir.dt.int32
AX = mybir.AxisListType
ALU = mybir.AluOpType
ACTF = mybir.ActivationFunctionType

V = 100000
N = 32
NCORES = 8
VC = V // NCORES          # 12500
P = 125                   # vertices per tile (partition dim)
T1 = VC // P              # 100 tiles per core
EXT = VC + 2              # stage-2 extended range (one halo vertex each side)
TA = T1 + 1               # 101 stage-2 gather tiles (last overlaps)

_CACHE = {}
TIMES = {}
_LAST_INPUTS = None


def _build_stage1(repeat=1):
    nc = bacc.Bacc("TRN2", target_bir_lowering=False, debug=False,
                   num_devices=NCORES)
    vp = nc.dram_tensor("vp", [V], F32, kind="ExternalInput")
    nb1 = nc.dram_tensor("nb1", [VC, N], I32, kind="ExternalInput")
    a1 = nc.dram_tensor("a1", [N + 1, N], F32, kind="ExternalInput")
    hsh = nc.dram_tensor("hsh", [VC], F32, kind="ExternalOutput")

    with tile.TileContext(nc) as tc:
        with (
            tc.tile_pool(name="const", bufs=1) as cp,
            tc.tile_pool(name="io", bufs=4) as iop,
            tc.tile_pool(name="work", bufs=4) as wp,
            tc.tile_pool(name="hc", bufs=1) as hcp,
            tc.tile_pool(name="ps", bufs=2, space="PSUM") as psp,
            tc.tile_pool(name="psb", bufs=1, space="PSUM") as psb,
        ):
            ident = cp.tile([128, 128], F32)
            make_identity(nc, ident[:])
            a1t = cp.tile([N + 1, N], F32)
            nc.sync.dma_start(a1t[:], a1[:])
            hcol = hcp.tile([P, T1], F32)

            rep = tc.For_i(0, repeat, 1) if repeat > 1 else None
            if rep is not None:
                rep.__enter__()
            t = 0
            for nb_batch in ([8] * (T1 // 8) + ([T1 % 8] if T1 % 8 else [])):
                it = iop.tile([P, N * nb_batch], I32, tag="idx")
                nc.sync.dma_start(
                    it[:].rearrange("p (b n) -> p b n", n=N),
                    nb1[P * t:P * (t + nb_batch), :].rearrange(
                        "(b p) n -> p b n", p=P))
                g = wp.tile([P, N * nb_batch], F32, tag="g")
                nc.gpsimd.indirect_dma_start(
                    out=g[:], out_offset=None, in_=vp[:, None],
                    in_offset=IndirectOffsetOnAxis(ap=it[:], axis=0))
                for b in range(nb_batch):
                    gtp = psp.tile([N, P], F32, tag="gt")
                    nc.tensor.transpose(gtp[:], g[:, N * b:N * (b + 1)],
                                        ident[:P, :P])
                    gt = wp.tile([N + 1, P], F32, tag="gts")
                    nc.vector.tensor_copy(gt[:N, :], gtp[:])
                    nc.vector.memset(gt[N:N + 1, :], 1.0)
                    c1p = psp.tile([P, N], F32, tag="c1")
                    nc.tensor.matmul(c1p[:], lhsT=gt[:], rhs=a1t[:],
                                     start=True, stop=True)
                    r = wp.tile([P, N], F32, tag="r")
                    nc.scalar.activation(r[:], c1p[:], ACTF.Relu)
                    nc.vector.reduce_sum(hcol[:, t:t + 1], r[:], axis=AX.X)
                    t += 1

            if rep is not None:
                rep.__exit__(None, None, None)
            htp = psb.tile([T1, P], F32)
            nc.tensor.transpose(htp[:], hcol[:], ident[:P, :P])
            hst = wp.tile([T1, P], F32, tag="hst")
            nc.vector.tensor_copy(hst[:], htp[:])
            nc.sync.dma_start(
                hsh[:].rearrange("(t p) -> t p", p=P), hst[:])
    nc.finalize()
    return nc


def _build_stage2(repeat=1, bench_internal_out=False):
    nc = bacc.Bacc("TRN2", target_bir_lowering=False, debug=False,
                   num_devices=NCORES)
    hp = nc.dram_tensor("hp", [V + 2], F32, kind="ExternalInput")
    nb2e = nc.dram_tensor("nb2e", [EXT, N], I32, kind="ExternalInput")
    mbig = nc.dram_tensor("mbig", [97, 1024], F32, kind="ExternalInput")
    w2k3 = nc.dram_tensor("w2k3", [96, 64], F32, kind="ExternalInput")
    wfcb = nc.dram_tensor("wfcb", [65, 512], F32, kind="ExternalInput")
    mask2 = nc.dram_tensor("mask2", [96, 2], F32, kind="ExternalInput")
    if bench_internal_out:
        out = nc.dram_tensor("out", [VC, 512], F32)
        tiny = nc.dram_tensor("tiny", [1, 1], F32, kind="ExternalOutput")
    else:
        out = nc.dram_tensor("out", [VC, 512], F32, kind="ExternalOutput")
        tiny = None

    with tile.TileContext(nc) as tc:
        with (
            tc.tile_pool(name="const", bufs=1) as cp,
            tc.tile_pool(name="strip", bufs=1) as sp,
            tc.tile_pool(name="io", bufs=4) as iop,
            tc.tile_pool(name="work", bufs=4) as wp,
            tc.tile_pool(name="big", bufs=3) as bp,
            tc.tile_pool(name="psc", bufs=2, space="PSUM") as psc,
            tc.tile_pool(name="pst", bufs=2, space="PSUM") as pst,
            tc.tile_pool(name="psl", bufs=2, space="PSUM") as psl,
        ):
            ident = cp.tile([128, 128], F32)
            make_identity(nc, ident[:])
            mbigt = cp.tile([97, 1024], F32)
            nc.sync.dma_start(mbigt[:], mbig[:])
            w2all = cp.tile([96, 64], F32)
            nc.sync.dma_start(w2all[:], w2k3[:])
            wfcbt = cp.tile([65, 512], F32)
            nc.sync.dma_start(wfcbt[:], wfcb[:])
            m2t = cp.tile([96, 2], F32)
            nc.sync.dma_start(m2t[:], mask2[:])

            strip3 = sp.tile([96, EXT], F32)

            def write_h2T(h2p_ap, ot):
                # strip3[32r+q, s] = h2T[q, c] for s = c - r  (c = ot..ot+125)
                for r in range(3):
                    lo = max(ot - r, 0)
                    nc.vector.tensor_copy(
                        strip3[32 * r:32 * (r + 1), lo:ot + P - r],
                        h2p_ap[:, lo - ot + r:P])

            rep = tc.For_i(0, repeat, 1) if repeat > 1 else None
            if rep is not None:
                rep.__enter__()

            def phase_a_batch(t0, nb_batch):
                # gather for nb_batch tiles; compute h2 into h2w groups of 4;
                # transpose each full 4-group into strip
                it = iop.tile([P, N * nb_batch], I32, tag="idx")
                nc.sync.dma_start(
                    it[:].rearrange("p (b n) -> p b n", n=N),
                    nb2e[P * t0:P * (t0 + nb_batch), :].rearrange(
                        "(b p) n -> p b n", p=P))
                tt = wp.tile([P, 3 * N * nb_batch], F32, tag="tt")
                nc.gpsimd.indirect_dma_start(
                    out=tt[:], out_offset=None, in_=hp[:, None],
                    in_offset=IndirectOffsetOnAxis(ap=it[:], axis=0))
                for b in range(nb_batch):
                    t = t0 + b
                    h2 = wp.tile([P, N], F32, tag="h2w")
                    compute_h2(tt[:, 96 * b:96 * (b + 1)], h2[:])
                    h2p = pst.tile([N, P], F32, tag="tp")
                    nc.tensor.transpose(h2p[:], h2[:], ident[:P, :P])
                    write_h2T(h2p[:], P * t)

            def compute_h2(tt_ap, h2_out):
                ttp = pst.tile([96, P], F32, tag="tp")
                nc.tensor.transpose(ttp[:], tt_ap, ident[:P, :P])
                tts = wp.tile([97, P], F32, tag="tts")
                nc.vector.tensor_copy(tts[:96, :], ttp[:])
                nc.vector.memset(tts[96:97, :], 1.0)
                cps = psc.tile([P, 1024], F32, tag="c")
                nc.tensor.matmul(cps[:, 0:512], lhsT=tts[:],
                                 rhs=mbigt[:, 0:512], start=True, stop=True)
                nc.tensor.matmul(cps[:, 512:1024], lhsT=tts[:],
                                 rhs=mbigt[:, 512:1024], start=True, stop=True)
                cr = bp.tile([P, 1024], F32, tag="cr")
                nc.scalar.activation(cr[:], cps[:], ACTF.Relu)
                nc.vector.reduce_sum(
                    h2_out, cr[:].rearrange("p (c j) -> p c j", j=32),
                    axis=AX.X)

            def phase_a_last(t):
                # single overlapping tile at the end (ot = EXT - P)
                ot = EXT - P
                it = iop.tile([P, N], I32, tag="idxl")
                nc.sync.dma_start(it[:], nb2e[ot:ot + P, :])
                tt = wp.tile([P, 3 * N], F32, tag="ttl")
                nc.gpsimd.indirect_dma_start(
                    out=tt[:], out_offset=None, in_=hp[:, None],
                    in_offset=IndirectOffsetOnAxis(ap=it[:], axis=0))
                h2 = wp.tile([P, N], F32, tag="h2l")
                compute_h2(tt[:], h2[:])
                h2p = pst.tile([N, P], F32, tag="tp")
                nc.tensor.transpose(h2p[:], h2[:], ident[:P, :P])
                write_h2T(h2p[:], ot)

            def phase_b_group(g):
                # 4 output tiles: vertices [500g, 500g+500)
                f2p = psl.tile([64, 500], F32, tag="lg")
                nc.tensor.matmul(f2p[:], lhsT=w2all[:],
                                 rhs=strip3[:, 500 * g:500 * (g + 1)],
                                 start=True, stop=True)
                f2s = wp.tile([65, 500], F32, tag="f2s")
                nc.vector.tensor_copy(f2s[:64, :], f2p[:])
                nc.vector.memset(f2s[64:65, :], 1.0)
                for b in range(4):
                    t = 4 * g + b
                    lgp = psl.tile([P, 512], F32, tag="lg")
                    nc.tensor.matmul(lgp[:], lhsT=f2s[:, P * b:P * (b + 1)],
                                     rhs=wfcbt[:], start=True, stop=True)
                    e = bp.tile([P, 512], F32, tag="e")
                    ssum = wp.tile([P, 1], F32, tag="ss")
                    nc.scalar.activation(e[:], lgp[:], ACTF.Exp,
                                         accum_out=ssum[:])
                    rinv = wp.tile([P, 1], F32, tag="ri")
                    nc.vector.reciprocal(rinv[:], ssum[:])
                    o = bp.tile([P, 512], F32, tag="o")
                    nc.vector.tensor_scalar(out=o[:], in0=e[:],
                                            scalar1=rinv[:], scalar2=None,
                                            op0=ALU.mult)
                    nc.sync.dma_start(out[bass.ts(t, P), :], o[:])

            done_a = 0
            done_b4 = 0      # phase-b groups emitted
            first = True
            for nb_batch in ([8] * (T1 // 8) + ([T1 % 8] if T1 % 8 else [])):
                phase_a_batch(done_a, nb_batch)
                done_a += nb_batch
                if first:
                    # mask left halo: strip col 0 lives at strip3[0:32, 0]
                    nc.vector.tensor_tensor(
                        out=strip3[0:32, 0:1], in0=strip3[0:32, 0:1],
                        in1=m2t[0:32, 0:1], op=ALU.mult)
                    first = False
                while 500 * (done_b4 + 1) + 2 <= P * done_a:
                    phase_b_group(done_b4)
                    done_b4 += 1
            phase_a_last(TA - 1)
            # mask right halo: strip col EXT-1 is read only at r=2, s=EXT-3
            nc.vector.tensor_tensor(
                out=strip3[64:96, EXT - 3:EXT - 2],
                in0=strip3[64:96, EXT - 3:EXT - 2],
                in1=m2t[64:96, 1:2], op=ALU.mult)
            while done_b4 < T1 // 4:
                phase_b_group(done_b4)
                done_b4 += 1

            if rep is not None:
                rep.__exit__(None, None, None)
            if tiny is not None:
                tz = wp.tile([1, 1], F32, tag="tz")
                nc.vector.memset(tz[:], 0.0)
                nc.sync.dma_start(tiny[:], tz[:])
    nc.finalize()
    return nc


def _host_mats(wv1, bv1, w1, b1, wv2, bv2, w2, b2, wfc, bfc):
    w1m = w1[:, 0, :].astype(np.float32)                    # [32, 3]
    a1 = np.zeros((N + 1, N), np.float32)                   # stage-1 conv
    for j in range(N):
        for dj in range(3):
            jp = j - 1 + dj
            if 0 <= jp < N:
                a1[jp, j] = wv1[dj]
    a1[N, :] = bv1[0]

    mbig = np.zeros((97, 1024), np.float32)
    cidx = np.arange(32) * 32
    for j in range(32):
        for dj in range(3):
            jp = j - 1 + dj
            if 0 <= jp < 32:
                for dk in range(3):
                    mbig[jp * 3 + dk, cidx + j] = wv2[dj] * w1m[:, dk] / 32.0
    for j in range(32):
        s = sum(wv2[dj] for dj in range(3) if 0 <= j - 1 + dj < 32)
        mbig[96, cidx + j] = bv2[0] + b1 * s

    w2k3 = np.zeros((96, 64), np.float32)
    for k in range(3):
        w2k3[32 * k:32 * k + 32, :] = w2[:, :, k].T / 32.0

    wfcb = np.zeros((65, 512), np.float32)
    wfcb[:64] = wfc.T
    wfcb[64] = bfc + wfc @ b2
    return a1, mbig, w2k3, wfcb


def kernel(vp, nb1, nb2, wv1, bv1, w1, b1, wv2, bv2, w2, b2, wfc, bfc):
    vp = np.ascontiguousarray(np.asarray(vp, dtype=np.float32))
    nb1 = np.ascontiguousarray(np.asarray(nb1).astype(np.int32))
    nb2 = np.ascontiguousarray(np.asarray(nb2).astype(np.int32))
    wv1 = np.asarray(wv1, np.float32); bv1 = np.asarray(bv1, np.float32)
    w1 = np.asarray(w1, np.float32); b1 = np.asarray(b1, np.float32)
    wv2 = np.asarray(wv2, np.float32); bv2 = np.asarray(bv2, np.float32)
    w2 = np.asarray(w2, np.float32); b2 = np.asarray(b2, np.float32)
    wfc = np.asarray(wfc, np.float32); bfc = np.asarray(bfc, np.float32)

    a1, mbig, w2k3, wfcb = _host_mats(wv1, bv1, w1, b1, wv2, bv2, w2, b2,
                                      wfc, bfc)

    if "s1" not in _CACHE:
        _CACHE["s1"] = _build_stage1()
    if "s2" not in _CACHE:
        _CACHE["s2"] = _build_stage2()

    core_ids = list(range(NCORES))

    # ---- stage 1 ----
    in1 = [{"vp": vp, "nb1": nb1[VC * c:VC * (c + 1)], "a1": a1}
           for c in range(NCORES)]
    t0 = time.time()
    res1 = run_bass_kernel_spmd(_CACHE["s1"], in1, core_ids=core_ids)
    TIMES["stage1_wall"] = time.time() - t0
    hp = np.zeros(V + 2, np.float32)
    for c in range(NCORES):
        hp[1 + VC * c:1 + VC * (c + 1)] = res1.results[c]["hsh"]

    # ---- stage 2 ----
    in2 = []
    for c in range(NCORES):
        vstart = VC * c
        nb2e = np.zeros((EXT, N), np.int32)
        lo = max(vstart - 1, 0)
        hi = min(vstart + VC + 1, V)
        nb2e[lo - (vstart - 1):hi - (vstart - 1)] = nb2[lo:hi]
        mask2 = np.ones((96, 2), np.float32)
        if c == 0:
            mask2[:, 0] = 0.0
        if c == NCORES - 1:
            mask2[:, 1] = 0.0
        in2.append({"hp": hp, "nb2e": nb2e, "mbig": mbig, "w2k3": w2k3,
                    "wfcb": wfcb, "mask2": mask2})
    global _LAST_INPUTS
    _LAST_INPUTS = (in1, in2)
    t0 = time.time()
    res2 = run_bass_kernel_spmd(_CACHE["s2"], in2, core_ids=core_ids)
    TIMES["stage2_wall"] = time.time() - t0
    return np.concatenate([res2.results[c]["out"] for c in range(NCORES)],
                          axis=0)
